# revision 1
# baseline (speedup 1.0000x reference)
"""DirectionalLoss Trainium2 kernel.

Computes total = 0.5*MSE + 0.5*(directional_loss + correlation_loss)/2 for
predictions/targets of shape [8192, 4096] f32, data-parallel over 8 cores
(1024 rows per core, 8 row-tiles of [128, 4096] each).

Engine split per row-tile (all f32; measured DVE passes are 1x = ~4.4us,
plain tensor_scalar is 2x = ~2.2us, ACT accum pass = 3.7us, GPSIMD TT =
~11us, PE is idle otherwise):
  ACT   : Square(x)+accum -> Sxx, Square(y)+accum -> Syy,
          Copy(x)+accum -> Sx,  Copy(y)+accum -> Sy
  DVE   : scalar_tensor_tensor (x+0)*y with accum -> Sxy,
          prod = pc*tc, mask1 = [prod>0], mask2 = [tc==0]
  GPSIMD: tc = y[:,1:]-y[:,:-1] every iter; pc = x-diff on 3 of 8 iters
  DVE   : pc on the other 5 iters
  PE    : ones^T @ mask chunks accumulated into a PSUM [1, 4096] counter
          (the directional count needs no per-row resolution, only a
          global sum, so the cross-partition matmul reduce is free
          parallelism on the idle TensorEngine)

Diff tiles are padded to the even width H with sentinel columns
(pc pad = +1, tc pad = -1 => prod pad = -1) so the full-width mask ops
contribute exactly 0 at the pad position.

Per-row corr/mse epilogue runs on [128, 8] stats tiles; each core outputs
per-partition partials [128, 2] plus the [1, 4096] count columns; the host
does the final tiny f64 reduction.
"""

import sys

for _p in ("/opt/trn_rl_repo", "/root/.axon_site/_ro/trn_rl_repo"):
    if _p not in sys.path:
        sys.path.insert(0, _p)

import numpy as np

import concourse.bass as bass
import concourse.tile as tile
from concourse import mybir
from concourse.bass_utils import run_bass_kernel_spmd

B_FULL = 8192
H = 4096
N_CORES = 8
ROWS_PER_CORE = B_FULL // N_CORES  # 1024
P = 128
N_TILES = ROWS_PER_CORE // P  # 8
EPSILON = 1e-6
MSE_WEIGHT = 0.5
DIRECTIONAL_WEIGHT = 0.5
MM_N = 512  # PSUM-bank-sized matmul free dim
N_CHUNKS = H // MM_N

F32 = mybir.dt.float32
Alu = mybir.AluOpType
Act = mybir.ActivationFunctionType

# iterations whose pc-diff runs on GPSIMD (the rest run on DVE)
PC_ON_GPSIMD = {0, 3, 6}


def _split_multiwait(nc, limit=1):
    """Hoist semaphore waits beyond `limit` into single-wait NoOps placed
    just before the owning instruction (same engine, so program order
    preserves the wait point). The walrus build in this container rejects
    instructions whose encoding has no room for >1 sync wait (e.g. the
    kernel-tail reset drain collects one wait per live semaphore)."""
    k = 0
    for f in nc.m.functions:
        for bb in f.blocks:
            insts = list(bb.instructions)
            out = []
            for ins in insts:
                si = ins.sync_info
                waits = list(si.on_wait) if si is not None and si.on_wait else []
                if len(waits) > limit:
                    spill, keep = waits[:-limit], waits[-limit:]
                    for w in spill:
                        k += 1
                        out.append(
                            mybir.InstNoOp(
                                name=f"waitnop-{k}",
                                engine=ins.engine,
                                sync_info=mybir.SyncInfo(on_wait=[w], on_update=[]),
                            )
                        )
                    ins.sync_info = mybir.SyncInfo(
                        on_wait=keep, on_update=list(si.on_update or [])
                    )
                out.append(ins)
            if len(out) != len(insts):
                bb.instructions = out


def build_bass(split_waits=True):
    nc = bass.Bass()
    x_d = nc.dram_tensor("x", [ROWS_PER_CORE, H], F32, kind="ExternalInput")
    y_d = nc.dram_tensor("y", [ROWS_PER_CORE, H], F32, kind="ExternalInput")
    stats_d = nc.dram_tensor("stats2", [P, 2], F32, kind="ExternalOutput")
    cnts_d = nc.dram_tensor("cnts", [1, H], F32, kind="ExternalOutput")

    with tile.TileContext(nc) as tc:
        with (
            tc.tile_pool(name="xin", bufs=2) as xin,
            tc.tile_pool(name="yin", bufs=2) as yin,
            tc.tile_pool(name="stats", bufs=1) as stats,
            tc.tile_pool(name="psum", bufs=1, space="PSUM") as psum_pool,
        ):
            sx = stats.tile([P, N_TILES], F32)
            sy = stats.tile([P, N_TILES], F32)
            sxx = stats.tile([P, N_TILES], F32)
            syy = stats.tile([P, N_TILES], F32)
            sxy = stats.tile([P, N_TILES], F32)
            ones = stats.tile([P, 1], mybir.dt.bfloat16)
            nc.vector.memset(ones[:], 1.0)

            BF16 = mybir.dt.bfloat16
            # bf16 copies of x/y (written by the ACT Copy passes that also
            # produce Sx/Sy) plus DMA-shifted aligned copies; diffs,
            # products and masks then all run in the DVE fast modes.
            # Shift-tile pad columns hold +/-1e30 sentinels so the pad
            # position of pc is > 0, of tc is < 0 (and != 0), and of
            # prod is -1e38 (finite in bf16): both masks contribute exactly 0 there.
            xyb_a = stats.tile([P, 2, H], BF16, tag="xybA")
            xyb_b = stats.tile([P, 2, H], BF16, tag="xybB")
            xyb_bufs = [xyb_a, xyb_b]
            pc_t = stats.tile([P, H], BF16)
            prod_t = stats.tile([P, H], BF16)
            tcd_t = stats.tile([P, H], BF16)
            mask_a = stats.tile([P, H], BF16, tag="maskA")
            mask_b = stats.tile([P, H], BF16, tag="maskB")
            dead_f32 = stats.tile([P, 1], F32)
            # sentinel pads: pc[+1e19] * tc[-1e19] = prod[-1e38]; is_gt and
            # is_eq both contribute 0 at the pad position
            nc.vector.memset(pc_t[:, H - 1 : H], 1.0e19)
            nc.vector.memset(tcd_t[:, H - 1 : H], -1.0e19)

            psum_cnt = psum_pool.tile([1, H], F32)

            for i in range(N_TILES):
                xt = xin.tile([P, H], F32)
                yt = yin.tile([P, H], F32)
                nc.sync.dma_start(out=xt[:], in_=x_d[i * P : (i + 1) * P, :])
                nc.sync.dma_start(out=yt[:], in_=y_d[i * P : (i + 1) * P, :])

                xyb = xyb_bufs[i % 2]
                xb = xyb[:, 0, :]
                yb = xyb[:, 1, :]

                # Sxy: out = (x+0)*y into a dead tile, accum_out = sum(x*y)
                nc.vector.scalar_tensor_tensor(
                    out=dead_f32.broadcast_to([P, H]),
                    in0=xt[:],
                    scalar=0.0,
                    in1=yt[:],
                    op0=Alu.add,
                    op1=Alu.mult,
                    accum_out=sxy[:, i : i + 1],
                )

                # row sums of x, y, x^2, y^2 on the scalar engine. Each op
                # needs a full-width `out` it will never be read from; a
                # 0-stride AP over a private [P,1] tile keeps every ACT
                # instruction down to a single sync wait (its input DMA) —
                # the Activation encoding has no room for more.
                def act_dead(tag):
                    t = stats.tile([P, 1], F32, tag=tag)
                    return t.broadcast_to([P, H])

                nc.scalar.activation(
                    out=xb[:], in_=xt[:], func=Act.Copy,
                    accum_out=sx[:, i : i + 1],
                )
                nc.scalar.activation(
                    out=yb[:], in_=yt[:], func=Act.Copy,
                    accum_out=sy[:, i : i + 1],
                )
                nc.scalar.activation(
                    out=act_dead(f"dsxx{i}"), in_=xt[:], func=Act.Square,
                    accum_out=sxx[:, i : i + 1],
                )
                nc.scalar.activation(
                    out=act_dead(f"dsyy{i}"), in_=yt[:], func=Act.Square,
                    accum_out=syy[:, i : i + 1],
                )

                # diffs straight off the bf16 copies: the one-element
                # offset read still runs in the DVE 2x mode (HW-verified)
                nc.vector.tensor_tensor(
                    out=pc_t[:, : H - 1], in0=xb[:, 1:], in1=xb[:, : H - 1],
                    op=Alu.subtract,
                )
                nc.vector.tensor_tensor(
                    out=tcd_t[:, : H - 1], in0=yb[:, 1:], in1=yb[:, : H - 1],
                    op=Alu.subtract,
                )
                nc.vector.tensor_tensor(
                    out=prod_t[:], in0=pc_t[:], in1=tcd_t[:], op=Alu.mult
                )
                nc.vector.tensor_scalar(
                    out=mask_a[:], in0=prod_t[:], scalar1=0.0, scalar2=None,
                    op0=Alu.is_gt,
                )
                nc.vector.tensor_scalar(
                    out=mask_b[:], in0=tcd_t[:], scalar1=0.0, scalar2=None,
                    op0=Alu.is_equal,
                )
                # PE: column-reduce both masks into the PSUM counter
                for c in range(N_CHUNKS):
                    nc.tensor.matmul(
                        psum_cnt[:, c * MM_N : (c + 1) * MM_N],
                        ones[:],
                        mask_a[:, c * MM_N : (c + 1) * MM_N],
                        start=(i == 0),
                        stop=False,
                    )
                for c in range(N_CHUNKS):
                    nc.tensor.matmul(
                        psum_cnt[:, c * MM_N : (c + 1) * MM_N],
                        ones[:],
                        mask_b[:, c * MM_N : (c + 1) * MM_N],
                        start=False,
                        stop=(i == N_TILES - 1),
                    )

            # ---- epilogue ----
            ep = stats
            sxsx = ep.tile([P, N_TILES], F32)
            sysy = ep.tile([P, N_TILES], F32)
            sxsy = ep.tile([P, N_TILES], F32)
            nc.vector.tensor_tensor(out=sxsx[:], in0=sx[:], in1=sx[:], op=Alu.mult)
            nc.vector.tensor_tensor(out=sysy[:], in0=sy[:], in1=sy[:], op=Alu.mult)
            nc.vector.tensor_tensor(out=sxsy[:], in0=sx[:], in1=sy[:], op=Alu.mult)

            ax = ep.tile([P, N_TILES], F32)
            ay = ep.tile([P, N_TILES], F32)
            nc.vector.scalar_tensor_tensor(
                out=ax[:], in0=sxsx[:], scalar=-1.0 / H, in1=sxx[:],
                op0=Alu.mult, op1=Alu.add,
            )
            nc.vector.scalar_tensor_tensor(
                out=ay[:], in0=sysy[:], scalar=-1.0 / H, in1=syy[:],
                op0=Alu.mult, op1=Alu.add,
            )
            sdx = ep.tile([P, N_TILES], F32)
            sdy = ep.tile([P, N_TILES], F32)
            nc.scalar.activation(
                out=sdx[:], in_=ax[:], func=Act.Sqrt, scale=1.0 / (H - 1)
            )
            nc.scalar.activation(
                out=sdy[:], in_=ay[:], func=Act.Sqrt, scale=1.0 / (H - 1)
            )
            nc.vector.tensor_scalar(
                out=sdx[:], in0=sdx[:], scalar1=EPSILON, scalar2=None, op0=Alu.add
            )
            nc.vector.tensor_scalar(
                out=sdy[:], in0=sdy[:], scalar1=EPSILON, scalar2=None, op0=Alu.add
            )
            den = ep.tile([P, N_TILES], F32)
            nc.vector.tensor_tensor(out=den[:], in0=sdx[:], in1=sdy[:], op=Alu.mult)
            rden = ep.tile([P, N_TILES], F32)
            nc.vector.reciprocal(out=rden[:], in_=den[:])

            num = ep.tile([P, N_TILES], F32)
            nc.vector.scalar_tensor_tensor(
                out=num[:], in0=sxsy[:], scalar=-1.0 / H, in1=sxy[:],
                op0=Alu.mult, op1=Alu.add,
            )
            corr = ep.tile([P, N_TILES], F32)
            nc.vector.scalar_tensor_tensor(
                out=corr[:], in0=num[:], scalar=1.0 / H, in1=rden[:],
                op0=Alu.mult, op1=Alu.mult,
            )

            stat2 = ep.tile([P, 2], F32)
            dead8 = ep.tile([P, N_TILES], F32)
            # col 0: per-partition sum of corr
            nc.vector.tensor_scalar(
                out=dead8[:], in0=corr[:], scalar1=0.0, scalar2=None,
                op0=Alu.add, op1=Alu.add, accum_out=stat2[:, 0:1],
            )
            # col 1: per-partition sum of (Sxx + Syy - 2*Sxy)
            t_m = ep.tile([P, N_TILES], F32)
            nc.vector.scalar_tensor_tensor(
                out=t_m[:], in0=sxy[:], scalar=-2.0, in1=sxx[:],
                op0=Alu.mult, op1=Alu.add,
            )
            dead8b = ep.tile([P, N_TILES], F32)
            nc.vector.scalar_tensor_tensor(
                out=dead8b[:], in0=t_m[:], scalar=0.0, in1=syy[:],
                op0=Alu.add, op1=Alu.add, accum_out=stat2[:, 1:2],
            )
            nc.sync.dma_start(out=stats_d[:], in_=stat2[:])

            # count columns: PSUM -> SBUF -> DRAM
            sb_cnt = ep.tile([1, H], F32)
            nc.vector.tensor_copy(out=sb_cnt[:], in_=psum_cnt[:])
            nc.sync.dma_start(out=cnts_d[:], in_=sb_cnt[:])

    if split_waits:
        _split_multiwait(nc)
    return nc


_NC_CACHE = None


def _get_nc():
    global _NC_CACHE
    if _NC_CACHE is None:
        _NC_CACHE = build_bass()
    return _NC_CACHE


def run_cores(predictions, targets, **kwargs):
    """Run the SPMD kernel; returns (per-core result dicts, BassKernelResults)."""
    nc = _get_nc()
    preds = np.ascontiguousarray(predictions, dtype=np.float32)
    targs = np.ascontiguousarray(targets, dtype=np.float32)
    in_maps = [
        {
            "x": preds[c * ROWS_PER_CORE : (c + 1) * ROWS_PER_CORE],
            "y": targs[c * ROWS_PER_CORE : (c + 1) * ROWS_PER_CORE],
        }
        for c in range(N_CORES)
    ]
    res = run_bass_kernel_spmd(nc, in_maps, core_ids=list(range(N_CORES)), **kwargs)
    return res.results, res


def _combine(outs):
    corr_sum = 0.0
    mse_sum = 0.0
    cnt_sum = 0.0
    for o in outs:
        s = o["stats2"].astype(np.float64)
        corr_sum += s[:, 0].sum()
        mse_sum += s[:, 1].sum()
        cnt_sum += o["cnts"].astype(np.float64).sum()
    mse = mse_sum / (B_FULL * H)
    directional_loss = 1.0 - cnt_sum / (B_FULL * (H - 1))
    correlation_loss = (B_FULL - corr_sum) / (2.0 * B_FULL)
    dir_combined = (directional_loss + correlation_loss) / 2.0
    total = MSE_WEIGHT * mse + DIRECTIONAL_WEIGHT * dir_combined
    return np.float32(total)


def kernel(predictions, targets):
    outs, _ = run_cores(predictions, targets)
    return np.asarray(_combine(outs))

